# revision 1
# baseline (speedup 1.0000x reference)
"""Trainium2 Bass kernel for 2-head causal self-attention.

Problem: embedded [B=4, S=2048, E=1024], Wq/Wk/Wv [H=2, E, HD=512].
out[b, s, h*HD:(h+1)*HD] = softmax(causal(Q K^T / sqrt(HD))) @ V for head h.

Sharding: 8 (b, h) pairs -> 8 cores, one pair each (perfect SPMD balance).

Per-core dataflow (phase 1 bf16 inputs, attention float32r; both 1 cyc/row):
  - Host passes X^T (so E is on the partition/contraction axis directly).
  - Phase 1: QT[d,q], KT[d,q] (W stationary, X^T moving) and V[k,d]
    (X^T stationary, W moving), q/k pipelined in 512-wide chunks.
  - Phase 2: scores are computed TRANSPOSED: scoresT[k, q-chunk] =
    (KT tile).T @ QT. After exp, the attnT tile [k, q] is exactly the
    stationary operand needed for ctx[q, d] += attnT.T @ V[k, d] --
    no on-device transpose of the attention matrix is ever needed.
    Softmax denominators accumulate transposed ([2, q] = ones.T @ attnT)
    and are rotated back to per-partition scalars with tiny contraction-
    dim-1 f32 matmuls. Causal masking
    is a 0/1 multiply on the 4 diagonal-block patterns (host constant);
    strictly-above-diagonal blocks are skipped entirely (~2x FLOP save).
"""

import ml_dtypes
import numpy as np

import concourse.bass as bass
import concourse.mybir as mybir
from concourse import bacc
import concourse.tile as tile
from concourse import bass_utils

B, S, E, H, HD = 4, 2048, 1024, 2, 512
P = 128
EO = E // P          # 8 e-tiles (contraction for QKV)
DT = HD // P         # 4 d-tiles (contraction for scores)
NKT = S // P         # 16 k-tiles
NSUP = S // 512      # 4 q super-tiles (512 wide)
SCALE = float(HD) ** -0.5
F32 = mybir.dt.float32
F32R = mybir.dt.float32r
EXP = mybir.ActivationFunctionType.Exp
BF16 = mybir.dt.bfloat16

_NC = None


def _body(tc, xt_d, wq_d, wk_d, wv_d, mask_d, ones_d, out_d):
    nc = tc.nc

    import contextlib

    with contextlib.ExitStack() as ctx:
        per = ctx.enter_context(tc.tile_pool(name="per", bufs=1))
        # Persistent SBUF: QT/KT as [d_inner=128, d_tile, q], V as [k_inner, k_tile, d]
        qt = per.tile([P, DT, S], F32R)
        kt = per.tile([P, DT, S], F32R)
        v = per.tile([P, NKT, HD], F32R)
        mask_sb = per.tile([P, 4, 512], F32R)
        ones = per.tile([P, 2], F32R)

        # ---------------- Phase 1: QT, KT, V projections ----------------
        # DMA order matters for the pipeline head: interleave wq/xc0 chunks
        # so the first QT accumulation can chase the DMAs sub-tile by
        # sub-tile instead of waiting behind all of wk/wv/mask. (DMAs are
        # split per sub-tile anyway: a single big DMA fans out across many
        # HW-DGE queues and the consuming matmul exceeds its sync-wait
        # slot limit.)
        with (
            tc.tile_pool(name="wpool", bufs=1) as wpool,
            tc.tile_pool(name="xpool", bufs=2) as xpool,
            tc.tile_pool(name="ps1", bufs=4, space="PSUM") as ps1,
        ):
            wq_sb = wpool.tile([P, EO, HD], BF16)
            wk_sb = wpool.tile([P, EO, HD], BF16)
            wv_sb = wpool.tile([P, EO, HD], BF16)
            xt_r = xt_d.rearrange("(eo p) q -> p eo q", p=P)

            xc0 = xpool.tile([P, EO, 512], BF16, tag="xc", name="xc")
            wq_r = wq_d.rearrange("(eo p) d -> p eo d", p=P)
            for eo in range(EO):
                nc.sync.dma_start(out=wq_sb[:, eo, :], in_=wq_r[:, eo, :])
                nc.sync.dma_start(out=xc0[:, eo, :], in_=xt_r[:, eo, 0:512])
            wk_r = wk_d.rearrange("(eo p) d -> p eo d", p=P)
            wv_r = wv_d.rearrange("(eo p) d -> p eo d", p=P)
            for eo in range(EO):
                nc.sync.dma_start(out=wk_sb[:, eo, :], in_=wk_r[:, eo, :])
                nc.sync.dma_start(out=wv_sb[:, eo, :], in_=wv_r[:, eo, :])
            nc.sync.dma_start(out=ones, in_=ones_d)
            for r in range(4):
                nc.sync.dma_start(out=mask_sb[:, r, :], in_=mask_d[r, :, :])

            for qc in range(4):  # 512-wide q/k chunk
                if qc == 0:
                    xc = xc0
                else:
                    xc = xpool.tile([P, EO, 512], BF16, tag="xc", name="xc")
                    for eo in range(EO):
                        nc.sync.dma_start(
                            out=xc[:, eo, :],
                            in_=xt_r[:, eo, qc * 512 : (qc + 1) * 512],
                        )

                # QT / KT: out[d_tile, q-chunk] = sum_e W[e, d].T @ XT[e, q]
                for w_sb, dst, eng in ((wq_sb, qt, "s"), (wk_sb, kt, "v")):
                    for dm in range(DT):
                        ps = ps1.tile([P, 512], F32, tag="ps")
                        for eo in range(EO):
                            nc.tensor.matmul(
                                ps,
                                lhsT=w_sb[:, eo, dm * P : (dm + 1) * P],
                                rhs=xc[:, eo, :],
                                start=(eo == 0),
                                stop=(eo == EO - 1),
                            )
                        dslice = dst[:, dm, qc * 512 : (qc + 1) * 512]
                        if eng == "s":
                            nc.scalar.copy(dslice, ps)
                        else:
                            nc.vector.tensor_copy(dslice, ps)

                # V: out[k_tile, d] = sum_e XT[e, k].T @ Wv[e, d]
                for ki in range(4):
                    kg = qc * 4 + ki
                    ps = ps1.tile([P, 512], F32, tag="ps")
                    for eo in range(EO):
                        nc.tensor.matmul(
                            ps,
                            lhsT=xc[:, eo, ki * P : (ki + 1) * P],
                            rhs=wv_sb[:, eo, :],
                            start=(eo == 0),
                            stop=(eo == EO - 1),
                        )
                    nc.vector.tensor_copy(v[:, kg, :], ps)

        # ---------------- Phase 2: attention ----------------
        with (
            tc.tile_pool(name="apool", bufs=2) as apool,
            tc.tile_pool(name="opool", bufs=3) as opool,
            tc.tile_pool(name="pss", bufs=2, space="PSUM") as pss,
            tc.tile_pool(name="psc", bufs=1, space="PSUM") as psc,
        ):
            for M in range(NSUP):  # q super-tile: q in [512M, 512(M+1))
                at = apool.tile([P, NKT, 512], F32R, tag="at")
                ctx_ps = [
                    psc.tile([P, HD], F32, tag=f"ctx{s}", name=f"ctx_ps{s}")
                    for s in range(4)
                ]
                # Row-sums accumulated TRANSPOSED [2, q=512] with a ones
                # stationary (full-width moving operand: 1 matmul per k-tile
                # instead of 4 tiny ones), then rotated to [q, 1] per
                # subtile via tiny contraction-dim-1 f32 matmuls.
                rsT_ps = psc.tile([2, 512], F32, tag="rsT")
                rsc_ps = psc.tile([P, 4], F32, tag="rsc")
                njt = 4 * M + 4  # causal: k-tiles 0 .. 4M+3
                for j in range(njt):
                    r = j - 4 * M
                    # Diagonal-zone tiles: cols < 128r are fully masked; skip
                    # what we can while keeping moving width >= 256 (fp32r
                    # runs 4x slower below 256).
                    off = min(P * r, 256) if r > 0 else 0
                    ps = pss.tile([P, 512], F32, tag="s")
                    for dt_i in range(DT):
                        nc.tensor.matmul(
                            ps[:, off:512],
                            lhsT=kt[:, dt_i, j * P : (j + 1) * P],
                            rhs=qt[:, dt_i, M * 512 + off : (M + 1) * 512],
                            start=(dt_i == 0),
                            stop=(dt_i == DT - 1),
                        )
                    a_j = at[:, j, off:512]
                    # attnT[k, q] = exp(scoresT / sqrt(hd)); masking after.
                    nc.scalar.activation(a_j, ps[:, off:512], EXP, scale=SCALE)
                    if r >= 0:  # diagonal-zone: zero invalid (q < k) cols
                        nc.vector.tensor_mul(a_j, a_j, mask_sb[:, r, off:512])
                    nc.tensor.matmul(
                        rsT_ps[:, off:512],
                        lhsT=ones,
                        rhs=a_j,
                        start=(j == 0),
                        stop=(j == njt - 1),
                    )
                # Rotate row-sums: [1, 512] row -> four [128, 1] columns via
                # matmuls with contraction dim 1 (plain f32: dst width 1 is
                # illegal for fp32r).
                rs_row = opool.tile([2, 512], F32, tag="rsrow")
                nc.vector.tensor_copy(rs_row[0:1, :], rsT_ps[0:1, :])
                one_f32 = ones[0:1, 0:1].bitcast(F32)
                for s in range(4):
                    nc.tensor.matmul(
                        rsc_ps[:, s : s + 1],
                        lhsT=rs_row[0:1, s * P : (s + 1) * P],
                        rhs=one_f32,
                        start=True,
                        stop=True,
                    )
                rinv4 = opool.tile([P, 4], F32, tag="rinv")
                nc.vector.reciprocal(rinv4, rsc_ps)
                # ctx[q_sub, d] += attnT_tile.T @ V. NOTE: hardware start=True
                # invalidates has_written for the WHOLE psum bank, so only one
                # accumulation group may be open per bank at a time (each ctx
                # subtile owns its own bank). s descending so the longest
                # chain retires first.
                for s in (3, 2, 1, 0):
                    for j in range(4 * M + s + 1):
                        nc.tensor.matmul(
                            ctx_ps[s],
                            lhsT=at[:, j, s * P : (s + 1) * P],
                            rhs=v[:, j, :],
                            start=(j == 0),
                            stop=(j == 4 * M + s),
                        )
                    o_sb = opool.tile([P, HD], F32, tag="o")
                    nc.vector.tensor_scalar_mul(o_sb, ctx_ps[s], rinv4[:, s : s + 1])
                    row0 = M * 512 + s * P
                    nc.sync.dma_start(out=out_d[row0 : row0 + P, :], in_=o_sb)


def _build_nc():
    nc = bacc.Bacc("TRN2", target_bir_lowering=False, debug=False, num_devices=8)
    xt_d = nc.dram_tensor("xt", [E, S], BF16, kind="ExternalInput")
    wq_d = nc.dram_tensor("wq", [E, HD], BF16, kind="ExternalInput")
    wk_d = nc.dram_tensor("wk", [E, HD], BF16, kind="ExternalInput")
    wv_d = nc.dram_tensor("wv", [E, HD], BF16, kind="ExternalInput")
    mask_d = nc.dram_tensor("mask", [4, P, 512], F32R, kind="ExternalInput")
    ones_d = nc.dram_tensor("ones", [P, 2], F32R, kind="ExternalInput")
    out_d = nc.dram_tensor("out", [S, HD], F32, kind="ExternalOutput")
    with tile.TileContext(nc) as tc:
        _body(tc, xt_d.ap(), wq_d.ap(), wk_d.ap(), wv_d.ap(), mask_d.ap(), ones_d.ap(), out_d.ap())
    nc.compile()
    return nc


def _mask_np():
    # mask[r][k_local, q_local] = 1 iff q_local >= 128*r + k_local
    q = np.arange(512)[None, :]
    k = np.arange(P)[:, None]
    return np.stack(
        [(q >= (P * r + k)).astype(np.float32) for r in range(4)], axis=0
    )


def _in_maps(embedded, Wq, Wk, Wv):
    embedded = np.asarray(embedded, dtype=np.float32)
    Wq = np.asarray(Wq, dtype=np.float32)
    Wk = np.asarray(Wk, dtype=np.float32)
    Wv = np.asarray(Wv, dtype=np.float32)
    mask = _mask_np()
    in_maps = []
    for core in range(8):
        b, h = divmod(core, 2)
        in_maps.append(
            {
                "xt": np.ascontiguousarray(embedded[b].T).astype(ml_dtypes.bfloat16),
                "wq": np.ascontiguousarray(Wq[h]).astype(ml_dtypes.bfloat16),
                "wk": np.ascontiguousarray(Wk[h]).astype(ml_dtypes.bfloat16),
                "wv": np.ascontiguousarray(Wv[h]).astype(ml_dtypes.bfloat16),
                "mask": mask,
                "ones": np.ones((P, 2), np.float32),
            }
        )
    return in_maps


def _gather(results):
    out = np.empty((B, S, H * HD), np.float32)
    for core in range(8):
        b, h = divmod(core, 2)
        out[b, :, h * HD : (h + 1) * HD] = results[core]["out"]
    return out


def _get_nc():
    global _NC
    if _NC is None:
        _NC = _build_nc()
    return _NC


def kernel(embedded, Wq, Wk, Wv):
    res = bass_utils.run_bass_kernel_spmd(
        _get_nc(), _in_maps(embedded, Wq, Wk, Wv), core_ids=list(range(8))
    )
    return _gather(res.results)


def kernel_traced(embedded, Wq, Wk, Wv):
    """Like kernel() but with NTFF tracing; returns (out, BassKernelResults)."""
    res = bass_utils.run_bass_kernel_spmd(
        _get_nc(), _in_maps(embedded, Wq, Wk, Wv), core_ids=list(range(8)), trace=True
    )
    return _gather(res.results), res



# revision 3
# speedup vs baseline: 1.1065x; 1.1065x over previous
"""Trainium2 Bass kernel for 2-head causal self-attention.

Problem: embedded [B=4, S=2048, E=1024], Wq/Wk/Wv [H=2, E, HD=512].
out[b, s, h*HD:(h+1)*HD] = softmax(causal(Q K^T / sqrt(HD))) @ V for head h.

Sharding: 8 (b, h) pairs -> 8 cores, one pair each (perfect SPMD balance).

Per-core dataflow (all matmul operands bf16: 1 cyc/row at any width):
  - Host passes X^T (so E is on the partition/contraction axis directly).
  - Phase 1: QT[d,q], KT[d,q] (W stationary, X^T moving) and V[k,d]
    (X^T stationary, W moving), q/k in 512-wide chunks. The first chunk
    runs eo-outer with 4 concurrent PSUM accumulation groups so compute
    chases the interleaved (wq, x) chunk DMAs as they land instead of
    stalling a full dm-chain on the last-arriving chunk.
  - Phase 2: scores are computed TRANSPOSED: scoresT[k, q-chunk] =
    (KT tile).T @ QT. After exp, the attnT tile [k, q] is exactly the
    stationary operand needed for ctx[q, d] += attnT.T @ V[k, d] --
    no on-device transpose of the attention matrix is ever needed.
    Diagonal-zone score tiles skip all fully-masked 128-col blocks
    (bf16 has no fp32r min-width penalty), so only the single 128x128
    block straddling the diagonal needs masking (one shared tril
    constant). Softmax denominators accumulate on the Vector engine
    (rsacc[k_inner, q] += attnT tile), then four 1-row matmuls per
    q-supertile (rsacc_slice.T @ ones[128,1]) rotate the partition-dim
    sums into per-partition scalars. ctx chains run s ascending so the
    last-exp'd attention tile is consumed last (hides exp latency).
"""

import ml_dtypes
import numpy as np

import concourse.bass as bass
import concourse.mybir as mybir
from concourse import bacc
import concourse.tile as tile
from concourse import bass_utils

B, S, E, H, HD = 4, 2048, 1024, 2, 512
P = 128
EO = E // P          # 8 e-tiles (contraction for QKV)
DT = HD // P         # 4 d-tiles (contraction for scores)
NKT = S // P         # 16 k-tiles
NSUP = S // 512      # 4 q super-tiles (512 wide)
SCALE = float(HD) ** -0.5
F32 = mybir.dt.float32
EXP = mybir.ActivationFunctionType.Exp
BF16 = mybir.dt.bfloat16

_NC = None


def _body(tc, xt_d, wq_d, wk_d, wv_d, mask_d, ones_d, out_d):
    nc = tc.nc

    import contextlib

    with contextlib.ExitStack() as ctx:
        per = ctx.enter_context(tc.tile_pool(name="per", bufs=1))
        # Persistent SBUF: QT/KT as [d_inner=128, d_tile, q], V as
        # [k_inner, k_tile, d], attnT as [k_inner, k_tile, q-chunk].
        qt = per.tile([P, DT, S], BF16)
        kt = per.tile([P, DT, S], BF16)
        v = per.tile([P, NKT, HD], BF16)
        at = per.tile([P, NKT, 512], BF16)
        mask_sb = per.tile([P, P], BF16)
        ones = per.tile([P, 1], BF16)
        rsacc = per.tile([P, 512], F32)
        rsb = per.tile([P, 512], BF16)

        # ---------------- Phase 1: QT, KT, V projections ----------------
        # DMA order matters for the pipeline head: interleave wq/xc0 chunks
        # so the first QT accumulation can chase the DMAs sub-tile by
        # sub-tile instead of waiting behind all of wk/wv. (DMAs are
        # split per sub-tile anyway: a single big DMA fans out across many
        # HW-DGE queues and the consuming matmul exceeds its sync-wait
        # slot limit.)
        with (
            tc.tile_pool(name="wpool", bufs=1) as wpool,
            tc.tile_pool(name="xpool", bufs=2) as xpool,
            tc.tile_pool(name="ps1", bufs=4, space="PSUM") as ps1,
        ):
            wq_sb = wpool.tile([P, EO, HD], BF16)
            wk_sb = wpool.tile([P, EO, HD], BF16)
            wv_sb = wpool.tile([P, EO, HD], BF16)
            xt_r = xt_d.rearrange("(eo p) q -> p eo q", p=P)

            xc0 = xpool.tile([P, EO, 512], BF16, tag="xc", name="xc")
            wq_r = wq_d.rearrange("(eo p) d -> p eo d", p=P)
            for eo in range(EO):
                nc.sync.dma_start(out=wq_sb[:, eo, :], in_=wq_r[:, eo, :])
                nc.sync.dma_start(out=xc0[:, eo, :], in_=xt_r[:, eo, 0:512])
            wk_r = wk_d.rearrange("(eo p) d -> p eo d", p=P)
            wv_r = wv_d.rearrange("(eo p) d -> p eo d", p=P)
            for eo in range(EO):
                nc.sync.dma_start(out=wk_sb[:, eo, :], in_=wk_r[:, eo, :])
            for eo in range(EO):
                nc.sync.dma_start(out=wv_sb[:, eo, :], in_=wv_r[:, eo, :])
            nc.sync.dma_start(out=ones, in_=ones_d)
            nc.sync.dma_start(out=mask_sb, in_=mask_d)

            for qc in range(4):  # 512-wide q/k chunk
                if qc == 0:
                    xc = xc0
                else:
                    xc = xpool.tile([P, EO, 512], BF16, tag="xc", name="xc")
                    for eo in range(EO):
                        nc.sync.dma_start(
                            out=xc[:, eo, :],
                            in_=xt_r[:, eo, qc * 512 : (qc + 1) * 512],
                        )

                # QT / KT: out[d_tile, q-chunk] = sum_e W[e, d].T @ XT[e, q]
                # V:       out[k_tile, d]      = sum_e XT[e, k].T @ Wv[e, d]
                # qc == 0 runs eo-outer (4 concurrent PSUM groups) to chase
                # the input DMAs; later chunks run dm-outer so the 4 banks
                # retire staggered and the PSUM->SBUF copies hide fully.
                def p1_matmul(ps, sub, eo, which):
                    if which == 0:
                        lhsT = wq_sb[:, eo, sub * P : (sub + 1) * P]
                        rhs = xc[:, eo, :]
                    elif which == 1:
                        lhsT = wk_sb[:, eo, sub * P : (sub + 1) * P]
                        rhs = xc[:, eo, :]
                    else:
                        lhsT = xc[:, eo, sub * P : (sub + 1) * P]
                        rhs = wv_sb[:, eo, :]
                    nc.tensor.matmul(
                        ps, lhsT=lhsT, rhs=rhs,
                        start=(eo == 0), stop=(eo == EO - 1),
                    )

                def p1_store(ps, sub, which):
                    if which == 0:
                        nc.scalar.copy(qt[:, sub, qc * 512 : (qc + 1) * 512], ps)
                    elif which == 1:
                        nc.vector.tensor_copy(kt[:, sub, qc * 512 : (qc + 1) * 512], ps)
                    else:
                        nc.vector.tensor_copy(v[:, qc * 4 + sub, :], ps)

                for which in range(3):
                    if qc == 0:
                        pss4 = [
                            ps1.tile([P, 512], F32, tag="ps", name=f"ps{i}")
                            for i in range(4)
                        ]
                        for eo in range(EO):
                            for sub in range(4):
                                p1_matmul(pss4[sub], sub, eo, which)
                        for sub in range(4):
                            p1_store(pss4[sub], sub, which)
                    else:
                        for sub in range(4):
                            ps = ps1.tile([P, 512], F32, tag="ps")
                            for eo in range(EO):
                                p1_matmul(ps, sub, eo, which)
                            p1_store(ps, sub, which)

        # ---------------- Phase 2: attention ----------------
        with (
            tc.tile_pool(name="opool", bufs=3) as opool,
            tc.tile_pool(name="pss", bufs=2, space="PSUM") as pss,
            tc.tile_pool(name="psc", bufs=4, space="PSUM") as psc,
            tc.tile_pool(name="psr", bufs=1, space="PSUM") as psr,
        ):
            for M in range(NSUP):  # q super-tile: q in [512M, 512(M+1))
                njt = 4 * M + 4  # causal: k-tiles 0 .. 4M+3
                for j in range(njt):
                    r = j - 4 * M
                    # Diagonal-zone tiles: cols < 128r are fully masked;
                    # skip them exactly (bf16 runs 1 cyc/row at any width).
                    off = P * r if r > 0 else 0
                    ps = pss.tile([P, 512], F32, tag="s")
                    for dt_i in range(DT):
                        nc.tensor.matmul(
                            ps[:, off:512],
                            lhsT=kt[:, dt_i, j * P : (j + 1) * P],
                            rhs=qt[:, dt_i, M * 512 + off : (M + 1) * 512],
                            start=(dt_i == 0),
                            stop=(dt_i == DT - 1),
                        )
                    a_j = at[:, j, off:512]
                    # attnT[k, q] = exp(scoresT / sqrt(hd)); mask after.
                    nc.scalar.activation(a_j, ps[:, off:512], EXP, scale=SCALE)
                    if r >= 0:
                        # only the 128x128 block straddling the diagonal is
                        # partially valid; cols > 128(r+1) are fully valid.
                        blk = at[:, j, off : off + P]
                        nc.vector.tensor_mul(blk, blk, mask_sb)
                    # Row-sums (over the k partition dim) accumulate on the
                    # Vector engine; rotated to per-partition scalars below.
                    if j == 0:
                        nc.vector.tensor_copy(rsacc, at[:, 0, :])
                    else:
                        nc.vector.tensor_add(
                            rsacc[:, off:], rsacc[:, off:], at[:, j, off:]
                        )

                # ctx[q_sub, d] += attnT_tile.T @ V. NOTE: hardware start=True
                # invalidates has_written for the WHOLE psum bank, so only one
                # accumulation group may be open per bank at a time (psc
                # bufs=4 gives each chain its own bank). s ascending so the
                # last-exp'd at tile (j=4M+3, needed only by chain s=3) is
                # consumed last.
                rs_ps = psr.tile([P, 4], F32, tag="rs")
                for s in range(4):
                    ctx_ps = psc.tile([P, HD], F32, tag="ctx")
                    for j in range(4 * M + s + 1):
                        nc.tensor.matmul(
                            ctx_ps,
                            lhsT=at[:, j, s * P : (s + 1) * P],
                            rhs=v[:, j, :],
                            start=(j == 0),
                            stop=(j == 4 * M + s),
                        )
                    if s == 0:
                        # rsacc is complete once the last exp+mask+add of
                        # this super-tile lands; rotate [k,q]-sums into
                        # per-q scalars with four 1-row matmuls.
                        nc.vector.tensor_copy(rsb, rsacc)
                        for t in range(4):
                            nc.tensor.matmul(
                                rs_ps[:, t : t + 1],
                                lhsT=rsb[:, t * P : (t + 1) * P],
                                rhs=ones,
                                start=True,
                                stop=True,
                            )
                        rinv4 = opool.tile([P, 4], F32, tag="rinv")
                        nc.vector.reciprocal(rinv4, rs_ps)
                    o_sb = opool.tile([P, HD], F32, tag="o")
                    nc.vector.tensor_scalar_mul(o_sb, ctx_ps, rinv4[:, s : s + 1])
                    row0 = M * 512 + s * P
                    nc.sync.dma_start(out=out_d[row0 : row0 + P, :], in_=o_sb)


def _build_nc():
    nc = bacc.Bacc("TRN2", target_bir_lowering=False, debug=False, num_devices=8)
    xt_d = nc.dram_tensor("xt", [E, S], BF16, kind="ExternalInput")
    wq_d = nc.dram_tensor("wq", [E, HD], BF16, kind="ExternalInput")
    wk_d = nc.dram_tensor("wk", [E, HD], BF16, kind="ExternalInput")
    wv_d = nc.dram_tensor("wv", [E, HD], BF16, kind="ExternalInput")
    mask_d = nc.dram_tensor("mask", [P, P], BF16, kind="ExternalInput")
    ones_d = nc.dram_tensor("ones", [P, 1], BF16, kind="ExternalInput")
    out_d = nc.dram_tensor("out", [S, HD], F32, kind="ExternalOutput")
    with tile.TileContext(nc) as tc:
        _body(tc, xt_d.ap(), wq_d.ap(), wk_d.ap(), wv_d.ap(), mask_d.ap(), ones_d.ap(), out_d.ap())
    nc.compile()
    return nc


def _mask_np():
    # tril incl diagonal: valid iff q_local >= k_local (within a 128 block)
    q = np.arange(P)[None, :]
    k = np.arange(P)[:, None]
    return (q >= k).astype(ml_dtypes.bfloat16)


def _in_maps(embedded, Wq, Wk, Wv):
    embedded = np.asarray(embedded, dtype=np.float32)
    Wq = np.asarray(Wq, dtype=np.float32)
    Wk = np.asarray(Wk, dtype=np.float32)
    Wv = np.asarray(Wv, dtype=np.float32)
    mask = _mask_np()
    in_maps = []
    for core in range(8):
        b, h = divmod(core, 2)
        in_maps.append(
            {
                "xt": np.ascontiguousarray(embedded[b].T).astype(ml_dtypes.bfloat16),
                "wq": np.ascontiguousarray(Wq[h]).astype(ml_dtypes.bfloat16),
                "wk": np.ascontiguousarray(Wk[h]).astype(ml_dtypes.bfloat16),
                "wv": np.ascontiguousarray(Wv[h]).astype(ml_dtypes.bfloat16),
                "mask": mask,
                "ones": np.ones((P, 1), ml_dtypes.bfloat16),
            }
        )
    return in_maps


def _gather(results):
    out = np.empty((B, S, H * HD), np.float32)
    for core in range(8):
        b, h = divmod(core, 2)
        out[b, :, h * HD : (h + 1) * HD] = results[core]["out"]
    return out


def _get_nc():
    global _NC
    if _NC is None:
        _NC = _build_nc()
    return _NC


def kernel(embedded, Wq, Wk, Wv):
    res = bass_utils.run_bass_kernel_spmd(
        _get_nc(), _in_maps(embedded, Wq, Wk, Wv), core_ids=list(range(8))
    )
    return _gather(res.results)


def kernel_traced(embedded, Wq, Wk, Wv):
    """Like kernel() but with NTFF tracing; returns (out, BassKernelResults)."""
    res = bass_utils.run_bass_kernel_spmd(
        _get_nc(), _in_maps(embedded, Wq, Wk, Wv), core_ids=list(range(8)), trace=True
    )
    return _gather(res.results), res


# revision 8
# speedup vs baseline: 1.1127x; 1.0056x over previous
"""Trainium2 Bass kernel for 2-head causal self-attention.

Problem: embedded [B=4, S=2048, E=1024], Wq/Wk/Wv [H=2, E, HD=512].
out[b, s, h*HD:(h+1)*HD] = softmax(causal(Q K^T / sqrt(HD))) @ V for head h.

Sharding: 8 (b, h) pairs -> 8 cores, one pair each (perfect SPMD balance).

Per-core dataflow (all matmul operands bf16: 1 cyc/row at any width):
  - Host passes X^T (so E is on the partition/contraction axis directly).
  - Phase 1: QT[d,q], KT[d,q] (W stationary, X^T moving) and V[k,d]
    (X^T stationary, W moving), q/k in 512-wide chunks. The first chunk
    runs eo-outer with 4 concurrent PSUM accumulation groups so compute
    chases the interleaved (wq, x) chunk DMAs as they land instead of
    stalling a full dm-chain on the last-arriving chunk.
  - Phase 2: scores are computed TRANSPOSED: scoresT[k, q-chunk] =
    (KT tile).T @ QT. After exp, the attnT tile [k, q] is exactly the
    stationary operand needed for ctx[q, d] += attnT.T @ V[k, d] --
    no on-device transpose of the attention matrix is ever needed.
    Diagonal-zone score tiles skip all fully-masked 128-col blocks
    (bf16 has no fp32r min-width penalty), so only the single 128x128
    block straddling the diagonal needs masking (one shared tril
    constant). Softmax denominators accumulate on the Vector engine
    (rsacc[k_inner, q] += attnT tile), then four 1-row matmuls per
    q-supertile (rsacc_slice.T @ ones[128,1]) rotate the partition-dim
    sums into per-partition scalars. ctx chains run s ascending so the
    last-exp'd attention tile is consumed last (hides exp latency).
"""

import ml_dtypes
import numpy as np

import concourse.bass as bass
import concourse.mybir as mybir
from concourse import bacc
import concourse.tile as tile
from concourse import bass_utils

B, S, E, H, HD = 4, 2048, 1024, 2, 512
P = 128
EO = E // P          # 8 e-tiles (contraction for QKV)
DT = HD // P         # 4 d-tiles (contraction for scores)
NKT = S // P         # 16 k-tiles
NSUP = S // 512      # 4 q super-tiles (512 wide)
SCALE = float(HD) ** -0.5
F32 = mybir.dt.float32
EXP = mybir.ActivationFunctionType.Exp
BF16 = mybir.dt.bfloat16

_NC = None


def _body(tc, xt_d, wq_d, wk_d, wv_d, mask_d, ones_d, out_d):
    nc = tc.nc

    import contextlib

    with contextlib.ExitStack() as ctx:
        per = ctx.enter_context(tc.tile_pool(name="per", bufs=1))
        # Persistent SBUF: QT/KT as [d_inner=128, d_tile, q], V as
        # [k_inner, k_tile, d], attnT as [k_inner, k_tile, q-chunk].
        qt = per.tile([P, DT, S], BF16)
        kt = per.tile([P, DT, S], BF16)
        v = per.tile([P, NKT, HD], BF16)
        at = per.tile([P, NKT, 512], BF16)
        mask_sb = per.tile([P, P], BF16)
        ones = per.tile([P, 1], BF16)
        rsacc = per.tile([P, 512], F32)
        rsb = per.tile([P, 512], BF16)
        scratch = per.tile([P, 1], BF16)

        # Warm the Exp activation table while the input DMAs stream (the
        # first real exp otherwise eats a ~1.3us ACT_TABLE_LOAD at the
        # phase 1 -> 2 boundary). Reads uninitialized SBUF; result unused.
        nc.scalar.activation(scratch, scratch, EXP, scale=1.0)

        # ---------------- Phase 1: QT, KT, V projections ----------------
        # Inputs are host-pretiled to [p, eo, ...] so every DMA writes
        # long contiguous per-partition runs (2-8 KB packets instead of
        # 1 KB: per-HW-queue drain rate roughly doubles) and each
        # dma_start trigger (~0.65us serial issue on the Sync queue)
        # moves 256 KB-1 MB instead of 128 KB. The head interleaves
        # wq/xc0 in 2-eo chunks so the first QT accumulation chases the
        # DMAs chunk by chunk.
        with (
            tc.tile_pool(name="wpool", bufs=1) as wpool,
            tc.tile_pool(name="xpool", bufs=2) as xpool,
            tc.tile_pool(name="ps1", bufs=4, space="PSUM") as ps1,
        ):
            wq_sb = wpool.tile([P, EO, HD], BF16)
            wk_sb = wpool.tile([P, EO, HD], BF16)
            wv_sb = wpool.tile([P, EO, HD], BF16)

            xc0 = xpool.tile([P, EO, 512], BF16, tag="xc", name="xc")
            for g in range(4):
                e0, e1 = 2 * g, 2 * g + 2
                nc.sync.dma_start(out=wq_sb[:, e0:e1, :], in_=wq_d[:, e0:e1, :])
                nc.sync.dma_start(out=xc0[:, e0:e1, :], in_=xt_d[0, :, e0:e1, :])
            for e0 in (0, 4):
                nc.sync.dma_start(
                    out=wk_sb[:, e0 : e0 + 4, :], in_=wk_d[:, e0 : e0 + 4, :]
                )
            for e0 in (0, 4):
                nc.sync.dma_start(
                    out=wv_sb[:, e0 : e0 + 4, :], in_=wv_d[:, e0 : e0 + 4, :]
                )
            nc.sync.dma_start(out=ones, in_=ones_d)
            nc.sync.dma_start(out=mask_sb, in_=mask_d)

            for qc in range(4):  # 512-wide q/k chunk
                if qc == 0:
                    xc = xc0
                else:
                    xc = xpool.tile([P, EO, 512], BF16, tag="xc", name="xc")
                    nc.sync.dma_start(out=xc, in_=xt_d[qc])

                # QT / KT: out[d_tile, q-chunk] = sum_e W[e, d].T @ XT[e, q]
                # V:       out[k_tile, d]      = sum_e XT[e, k].T @ Wv[e, d]
                # qc == 0 runs eo-outer (4 concurrent PSUM groups) to chase
                # the input DMAs; later chunks run dm-outer so the 4 banks
                # retire staggered and the PSUM->SBUF copies hide fully.
                def p1_matmul(ps, sub, eo, which):
                    if which == 0:
                        lhsT = wq_sb[:, eo, sub * P : (sub + 1) * P]
                        rhs = xc[:, eo, :]
                    elif which == 1:
                        lhsT = wk_sb[:, eo, sub * P : (sub + 1) * P]
                        rhs = xc[:, eo, :]
                    else:
                        lhsT = xc[:, eo, sub * P : (sub + 1) * P]
                        rhs = wv_sb[:, eo, :]
                    nc.tensor.matmul(
                        ps, lhsT=lhsT, rhs=rhs,
                        start=(eo == 0), stop=(eo == EO - 1),
                    )

                def p1_store(ps, sub, which):
                    if which == 0:
                        nc.scalar.copy(qt[:, sub, qc * 512 : (qc + 1) * 512], ps)
                    elif which == 1:
                        nc.vector.tensor_copy(kt[:, sub, qc * 512 : (qc + 1) * 512], ps)
                    else:
                        nc.vector.tensor_copy(v[:, qc * 4 + sub, :], ps)

                for which in range(3):
                    if qc == 0:
                        pss4 = [
                            ps1.tile([P, 512], F32, tag="ps", name=f"ps{i}")
                            for i in range(4)
                        ]
                        for eo in range(EO):
                            for sub in range(4):
                                p1_matmul(pss4[sub], sub, eo, which)
                        for sub in range(4):
                            p1_store(pss4[sub], sub, which)
                    else:
                        for sub in range(4):
                            ps = ps1.tile([P, 512], F32, tag="ps")
                            for eo in range(EO):
                                p1_matmul(ps, sub, eo, which)
                            p1_store(ps, sub, which)

        # ---------------- Phase 2: attention ----------------
        with (
            tc.tile_pool(name="opool", bufs=3) as opool,
            tc.tile_pool(name="pss", bufs=2, space="PSUM") as pss,
            tc.tile_pool(name="psc", bufs=4, space="PSUM") as psc,
            tc.tile_pool(name="psr", bufs=1, space="PSUM") as psr,
        ):
            for M in range(NSUP):  # q super-tile: q in [512M, 512(M+1))
                njt = 4 * M + 4  # causal: k-tiles 0 .. 4M+3
                for j in range(njt):
                    r = j - 4 * M
                    # Diagonal-zone tiles: cols < 128r are fully masked;
                    # skip them exactly (bf16 runs 1 cyc/row at any width).
                    off = P * r if r > 0 else 0
                    ps = pss.tile([P, 512], F32, tag="s")
                    for dt_i in range(DT):
                        nc.tensor.matmul(
                            ps[:, off:512],
                            lhsT=kt[:, dt_i, j * P : (j + 1) * P],
                            rhs=qt[:, dt_i, M * 512 + off : (M + 1) * 512],
                            start=(dt_i == 0),
                            stop=(dt_i == DT - 1),
                        )
                    a_j = at[:, j, off:512]
                    # attnT[k, q] = exp(scoresT / sqrt(hd)); mask after.
                    nc.scalar.activation(a_j, ps[:, off:512], EXP, scale=SCALE)
                    if r >= 0:
                        # only the 128x128 block straddling the diagonal is
                        # partially valid; cols > 128(r+1) are fully valid.
                        blk = at[:, j, off : off + P]
                        nc.vector.tensor_mul(blk, blk, mask_sb)
                    # Row-sums (over the k partition dim) accumulate on the
                    # Vector engine; rotated to per-partition scalars below.
                    if j == 0:
                        nc.vector.tensor_copy(rsacc, at[:, 0, :])
                    else:
                        nc.vector.tensor_add(
                            rsacc[:, off:], rsacc[:, off:], at[:, j, off:]
                        )

                # ctx[q_sub, d] += attnT_tile.T @ V. NOTE: hardware start=True
                # invalidates has_written for the WHOLE psum bank, so only one
                # accumulation group may be open per bank at a time (psc
                # bufs=4 gives each chain its own bank). s ascending so the
                # last-exp'd at tile (j=4M+3, needed only by chain s=3) is
                # consumed last. The four 1-row row-sum matmuls (rotating
                # the partition-dim sums into per-q scalars) are spread
                # between chains: late enough that the Vector-engine rsacc
                # accumulation has landed, and each one's ldweights hides
                # under the preceding chain's 512-row matmuls.
                rs_ps = psr.tile([P, 4], F32, tag="rs")
                rinv4 = opool.tile([P, 4], F32, tag="rinv")
                # tiny row-sum matmuls due after ctx chain s:
                tiny_after = {2: (0, 1), 3: (2, 3)} if M == 0 else {1: (0, 1), 2: (2,), 3: (3,)}
                pending = []

                def emit_tiny(t):
                    nc.tensor.matmul(
                        rs_ps[:, t : t + 1],
                        lhsT=rsb[:, t * P : (t + 1) * P],
                        rhs=ones,
                        start=True,
                        stop=True,
                    )
                    nc.vector.reciprocal(rinv4[:, t : t + 1], rs_ps[:, t : t + 1])

                for s in range(4):
                    ctx_ps = psc.tile([P, HD], F32, tag="ctx")
                    for j in range(4 * M + s + 1):
                        nc.tensor.matmul(
                            ctx_ps,
                            lhsT=at[:, j, s * P : (s + 1) * P],
                            rhs=v[:, j, :],
                            start=(j == 0),
                            stop=(j == 4 * M + s),
                        )
                    if s in tiny_after:
                        if min(tiny_after[s]) == 0:
                            nc.vector.tensor_copy(rsb, rsacc)
                        for t in tiny_after[s]:
                            emit_tiny(t)
                    pending.append((s, ctx_ps))
                    # scale+store every chain whose rinv is now available
                    ready = []
                    for ss, cps in pending:
                        if ss <= max(
                            (t for k, ts in tiny_after.items() if k <= s for t in ts),
                            default=-1,
                        ):
                            o_sb = opool.tile([P, HD], F32, tag="o", name="o_sb")
                            nc.vector.tensor_scalar_mul(o_sb, cps, rinv4[:, ss : ss + 1])
                            row0 = M * 512 + ss * P
                            nc.sync.dma_start(out=out_d[row0 : row0 + P, :], in_=o_sb)
                            ready.append((ss, cps))
                    for item in ready:
                        pending.remove(item)


def _build_nc():
    nc = bacc.Bacc("TRN2", target_bir_lowering=False, debug=False, num_devices=8)
    # host-pretiled layouts: xt[qc, p, eo, q'], w*[p, eo, d] so per-
    # partition DMA runs are contiguous (see phase 1 comment)
    xt_d = nc.dram_tensor("xt", [4, P, EO, 512], BF16, kind="ExternalInput")
    wq_d = nc.dram_tensor("wq", [P, EO, HD], BF16, kind="ExternalInput")
    wk_d = nc.dram_tensor("wk", [P, EO, HD], BF16, kind="ExternalInput")
    wv_d = nc.dram_tensor("wv", [P, EO, HD], BF16, kind="ExternalInput")
    mask_d = nc.dram_tensor("mask", [P, P], BF16, kind="ExternalInput")
    ones_d = nc.dram_tensor("ones", [P, 1], BF16, kind="ExternalInput")
    out_d = nc.dram_tensor("out", [S, HD], F32, kind="ExternalOutput")
    with tile.TileContext(nc) as tc:
        _body(tc, xt_d.ap(), wq_d.ap(), wk_d.ap(), wv_d.ap(), mask_d.ap(), ones_d.ap(), out_d.ap())
    nc.compile()
    return nc


def _mask_np():
    # tril incl diagonal: valid iff q_local >= k_local (within a 128 block)
    q = np.arange(P)[None, :]
    k = np.arange(P)[:, None]
    return (q >= k).astype(ml_dtypes.bfloat16)


def _tile_x(x):
    # x: [S, E] -> [qc, p, eo, q'] with x[qc*512+q', eo*128+p]
    return np.ascontiguousarray(
        x.reshape(4, 512, EO, P).transpose(0, 3, 2, 1)
    ).astype(ml_dtypes.bfloat16)


def _tile_w(w):
    # w: [E, HD] -> [p, eo, d] with w[eo*128+p, d]
    return np.ascontiguousarray(w.reshape(EO, P, HD).transpose(1, 0, 2)).astype(
        ml_dtypes.bfloat16
    )


def _in_maps(embedded, Wq, Wk, Wv):
    embedded = np.asarray(embedded, dtype=np.float32)
    Wq = np.asarray(Wq, dtype=np.float32)
    Wk = np.asarray(Wk, dtype=np.float32)
    Wv = np.asarray(Wv, dtype=np.float32)
    mask = _mask_np()
    in_maps = []
    for core in range(8):
        b, h = divmod(core, 2)
        in_maps.append(
            {
                "xt": _tile_x(embedded[b]),
                "wq": _tile_w(Wq[h]),
                "wk": _tile_w(Wk[h]),
                "wv": _tile_w(Wv[h]),
                "mask": mask,
                "ones": np.ones((P, 1), ml_dtypes.bfloat16),
            }
        )
    return in_maps


def _gather(results):
    out = np.empty((B, S, H * HD), np.float32)
    for core in range(8):
        b, h = divmod(core, 2)
        out[b, :, h * HD : (h + 1) * HD] = results[core]["out"]
    return out


def _get_nc():
    global _NC
    if _NC is None:
        _NC = _build_nc()
    return _NC


def kernel(embedded, Wq, Wk, Wv):
    res = bass_utils.run_bass_kernel_spmd(
        _get_nc(), _in_maps(embedded, Wq, Wk, Wv), core_ids=list(range(8))
    )
    return _gather(res.results)


def kernel_traced(embedded, Wq, Wk, Wv):
    """Like kernel() but with NTFF tracing; returns (out, BassKernelResults)."""
    res = bass_utils.run_bass_kernel_spmd(
        _get_nc(), _in_maps(embedded, Wq, Wk, Wv), core_ids=list(range(8)), trace=True
    )
    return _gather(res.results), res
